# revision 16
# baseline (speedup 1.0000x reference)
"""Trainium2 Bass kernel for Keras-style CTC batch loss.

Problem: y_pred [256, 256, 512] f32 softmax probs, y_true [256, 64] int64
labels (0=pad, blank=511). Output [256, 1] f32 negative log-likelihood.

Strategy (pure data parallel, 32 samples per core on 8 cores):
  - Stream y_pred in 16 loads of 1 MB: stage [128 part = 8 samples x 16
    chunks, 4 t x 512 c + zero pad]; every partition line is a contiguous
    8 KB HBM read (bwd t-reversal handled by the pstash-write AP + idx
    tables). fwd loads ride the SP HWDGE ring, bwd loads the ACT HWDGE
    ring, so the two rings drain in parallel at HBM rate.
  - GPSIMD ap_gather pulls 4x144 extended-state probabilities per
    partition (per-core idx = one sample, shared across its 16 chunk
    partitions); ACT casts to bf16 with the reference's +1e-7 fused; a
    collapse DMA lands rows into a [64 rows, t-major] probability store
    (2 tiles of 64 t-blocks each).
  - DVE runs the serial CTC recursion in linear space: 127 slots x 4
    bf16 tensor_tensor ops on [64, 132]. Rows 0..31: forward alpha from
    t=0..127. Rows 32..63: backward adjoint from t=255..128 with the
    state axis flipped about s=128 so both directions share identical
    shifted-view ops. Renorm bookkeeping is pure DVE (fused row-max via
    tensor_tensor_reduce + exponent-field bit math), so the recursion
    never waits on the ACT queue and overlaps the streaming phase.
  - Combine: one extra band apply on the forward rows, reversed dot with
    the backward rows, ln on ACT, negate, DMA out.

Self-contained: shapes/sharding hardcoded; no problem files read.
"""

import numpy as np
import ml_dtypes
from contextlib import ExitStack

import concourse.bass as bass
import concourse.tile as tile
from concourse import bacc, mybir
from concourse.bass_utils import run_bass_kernel_spmd

# ---------------- problem constants ----------------
B, T, C, L = 256, 256, 512, 64
S = 2 * L + 1          # 129 extended states
BLANK = C - 1          # 511
EPS = 1e-7
NCORES = 8
SPC = B // NCORES      # 32 samples per core
R = 2 * SPC            # 64 recursion rows (fwd + bwd)
W = 144                # gather width per t-step (mult of 16, >= S)
FD = 132               # recursion free size (states j = 0..131)
NSLOT = 127            # recursion slots
TQ = 4                 # t-steps packed per stage partition
NCHUNK = 16            # chunk partitions per sample in a stage
TL = TQ * NCHUNK       # 64 t-steps per load
NLOAD = 16             # 8 fwd + 8 bwd
ZCOL = TQ * C          # stage column with exact 0.0 (2048)
NELEM = ZCOL + 4       # stage width incl zero columns (2052)
W4 = TQ * W            # gather idxs per partition (576)
NPH = 2                # phase tiles per direction-merged store
PHW = TL * W           # phase tile width (9216)
RENORM_EVERY = 10
RENORM_SLOTS = tuple(range(3, NSLOT - 2, RENORM_EVERY))
# renorm target 2^36 (biased exp 163): must stay low enough that the FIRST
# renorm's scale 2^(163-eb) is f32-representable (init alpha ~1e-11 -> eb~91
# -> scale 2^72); 10-slot decay (~30 decades) keeps max ~7e-20, far above
# bf16 flush-to-zero even with ~13 decades of state spread.
RT_B = 163.0
RT_2B = 290.0
V1_RENORM = False  # debug: use the v1 ACT-exp renorm path
DEBUG_PH = False   # debug: dump phase stores to dram
LN2 = float(np.log(2.0))
F32 = mybir.dt.float32
BF16 = mybir.dt.bfloat16
I16 = mybir.dt.int16
U32 = mybir.dt.uint32
AF = mybir.ActivationFunctionType
ALU = mybir.AluOpType


# ---------------- host-side tables ----------------
def _ext_and_mask(labels):
    l = int(np.count_nonzero(labels))
    ext = np.full(S, BLANK, np.int64)
    ext[1::2] = np.asarray(labels, np.int64)
    m = np.zeros(S, np.float32)
    mm = (ext[2:] != BLANK) & (ext[2:] != ext[:-2])
    m[2:] = mm.astype(np.float32)
    return ext, m, l


def _wrap576(idx_row):
    """[W4] -> ap_gather wrapped layout [16, W4//16] for one core."""
    return np.asarray(idx_row, np.int16).reshape(W4 // 16, 16).T


def _core_tables(y_true_core):
    """Per-core device tables from labels [SPC, L].

    idxF/idxB: 4 tables [128, W4//16] int16 each (per 8-sample h-group);
    msrc [R, W] bf16 source-side skip mask; ind [R, W] bf16 init flags.
    """
    # per-sample single-t idx rows (within one 513-ish sub-table of 512 + zero)
    fwd_rows = np.full((SPC, W), ZCOL, np.int64)   # offsets into [0, NELEM)
    bwd_rows = np.full((SPC, W), ZCOL, np.int64)
    msrc = np.zeros((R, W), np.float32)
    ind = np.zeros((R, W), np.float32)
    for b in range(SPC):
        ext, m, l = _ext_and_mask(y_true_core[b])
        fwd_rows[b, :2 * l + 1] = ext[:2 * l + 1]
        msrc[b, : S - 2] = m[2:]
        ind[b, 0] = 1.0
        ind[b, 1] = 1.0
        r = SPC + b
        for i in range(S):
            s = 128 - i
            if s <= 2 * l:
                bwd_rows[b, i] = ext[s]
        sidx = 128 - np.arange(W)
        ok = (sidx >= 2) & (sidx <= S - 1)
        msrc[r, ok] = m[sidx[ok]]
        ind[r, 128 - 2 * l] = 1.0
        ind[r, 129 - 2 * l] = 1.0

    def tables(rows, rev):
        # rev: bwd stages hold 4 ascending t per partition while the stream
        # order is t-descending, so sub-position tr reads sub-table 3-tr.
        out = []
        for h in range(4):
            t = np.zeros((128, W4 // 16), np.int16)
            for g in range(8):
                b = 8 * h + g
                full = np.concatenate(
                    [np.where(rows[b] < ZCOL,
                              rows[b] + (TQ - 1 - tr if rev else tr) * C,
                              ZCOL)
                     for tr in range(TQ)])
                t[16 * g: 16 * g + 16, :] = _wrap576(full)
            out.append(t)
        return out

    return (tables(fwd_rows, False), tables(bwd_rows, True),
            msrc.astype(ml_dtypes.bfloat16),
            ind.astype(ml_dtypes.bfloat16))


# ---------------- device kernel ----------------
def _emit(nc):
    yp = nc.dram_tensor("yp", [SPC, T, C], F32, kind="ExternalInput")
    idxf_d = [nc.dram_tensor(f"idxf{h}", [128, W4 // 16], I16,
                             kind="ExternalInput") for h in range(4)]
    idxb_d = [nc.dram_tensor(f"idxb{h}", [128, W4 // 16], I16,
                             kind="ExternalInput") for h in range(4)]
    msrc_d = nc.dram_tensor("msrc", [R, W], BF16, kind="ExternalInput")
    ind_d = nc.dram_tensor("ind", [R, W], BF16, kind="ExternalInput")
    out_d = nc.dram_tensor("loss_out", [SPC, 1], F32, kind="ExternalOutput")

    with tile.TileContext(nc) as tc, ExitStack() as ctx:
        consts = ctx.enter_context(tc.tile_pool(name="consts", bufs=1))
        stage_p = ctx.enter_context(tc.tile_pool(name="stage", bufs=1))
        gout_p = ctx.enter_context(tc.tile_pool(name="gout", bufs=3))
        phase_p = ctx.enter_context(tc.tile_pool(name="phase", bufs=1))
        state_p = ctx.enter_context(tc.tile_pool(name="state", bufs=1))
        tmp_p = ctx.enter_context(tc.tile_pool(name="tmp", bufs=3))

        idxf = [consts.tile([128, W4 // 16], I16, name=f"idxf{h}")
                for h in range(4)]
        idxb = [consts.tile([128, W4 // 16], I16, name=f"idxb{h}")
                for h in range(4)]
        msrc = consts.tile([R, W], BF16)
        ind = consts.tile([R, W], BF16)
        # consts ride the gpsimd SWDGE queue so the sync/scalar HWDGE rings
        # start pulling y_pred immediately
        for h in range(4):
            nc.gpsimd.dma_start(idxf[h][:, :], idxf_d[h].ap())
            nc.gpsimd.dma_start(idxb[h][:, :], idxb_d[h].ap())
        nc.gpsimd.dma_start(msrc[:, :], msrc_d.ap())
        nc.gpsimd.dma_start(ind[:, :], ind_d.ap())

        # 13 stages (vs 16 loads) to fit f32 phase tiles in SBUF; loads
        # 13..15 reuse stages 0..2, whose gathers finish long before those
        # loads' descriptors are generated.
        NSTAGE = 13
        stages = [stage_p.tile([128, NELEM], F32, name=f"stage{i}")
                  for i in range(NSTAGE)]
        for st in stages:
            nc.vector.memset(st[:, ZCOL:NELEM], 0.0)

        # f32 phase stores, written straight from the gather output by
        # SBUF->SBUF DMAs (no dtype cast anywhere: DMA-cast runs ~16us/write
        # and an ACT-cast adds ~15us blocking cross-engine sem waits per
        # load). The recursion multiplies read f32 psl directly.
        phase = [phase_p.tile([R, PHW], F32, name=f"phase{j}")
                 for j in range(NPH)]

        # loads: (is_bwd, j, h); order fwd/bwd interleaved, j-major
        loads = []
        for j in range(NPH):
            for h in range(4):
                loads.append((0, j, h))
                loads.append((1, j, h))

        def src_ap(is_bwd, j, h):
            # Always a contiguous, t-ascending chunk range: 8KB/partition HBM
            # lines (the SDMA fast path). The bwd t-reversal happens in the
            # pstash-write AP (chunk dim) + idx tables (sub-t).
            blocks = yp.ap()[8 * h: 8 * h + 8, :, :].rearrange(
                "b (k t) c -> b k (t c)", t=TQ)
            lo = (T // TQ - NCHUNK * (j + 1)) if is_bwd else NCHUNK * j
            return blocks[:, lo: lo + NCHUNK, :]

        def issue_load(i):
            # fwd loads on the SP HWDGE ring, bwd on the ACT HWDGE ring: the
            # two rings drain in parallel, bounded only by HBM bandwidth.
            d, j, h = loads[i]
            eng = nc.scalar if d else nc.sync
            eng.dma_start(stages[i % NSTAGE][:, 0:ZCOL], src_ap(d, j, h))

        for i in range(NSTAGE):
            issue_load(i)

        for i, (d, j, h) in enumerate(loads):
            st = stages[i % NSTAGE]
            go = gout_p.tile([128, W4], F32, tag="gout")
            idx = (idxb if d else idxf)[h]
            nc.gpsimd.ap_gather(
                out_ap=go[:, :], in_ap=st[:, :], idxs_ap=idx[:, :],
                channels=128, num_elems=NELEM, d=1, num_idxs=W4)
            if i + NSTAGE < len(loads):
                # the load reusing this stage must be emitted AFTER its
                # gather so the WAR is in program order
                issue_load(i + NSTAGE)
            if DEBUG_PH and i == 0:
                dbg_st = nc.dram_tensor("dbg_st", [128, NELEM], F32,
                                        kind="ExternalOutput")
                dbg_go = nc.dram_tensor("dbg_go", [128, W4], F32,
                                        kind="ExternalOutput")
                nc.sync.dma_start(dbg_st.ap(), st[:, :])
                nc.sync.dma_start(dbg_go.ap(), go[:, :])
            # collapse straight into the phase tile: src partition (g, c) ->
            # dst row (d, h, g), col block c (fwd) or 15-c (bwd t-reversal;
            # stage chunk c holds t-block lo+c ascending, phase col p needs
            # t-block 63-16j-p = stage chunk 15-p). Same gpsimd queue as the
            # gather, so no cross-engine ping-pong.
            rbase = d * 32 + h * 8
            dst = phase[j][rbase: rbase + 8, :].rearrange(
                "g (c s) -> g c s", c=NCHUNK)
            if d:
                dst = dst[:, ::-1, :]
            # src [128, 576] flattens as (g, c, s), matching dst (g, c, s)
            nc.gpsimd.dma_start(dst, go[:, :])

        if DEBUG_PH:
            dbg = [nc.dram_tensor(f"dbg_ph{j}", [R, PHW], F32,
                                  kind="ExternalOutput") for j in range(NPH)]
            for j in range(NPH):
                nc.sync.dma_start(dbg[j].ap(), phase[j][:, :])

        # ---- recursion state ----
        bufA = state_p.tile([R, 2 + W], BF16)
        bufB = state_p.tile([R, 2 + W], BF16)
        racc = state_p.tile([R, 1], F32)
        mx = state_p.tile([R, 1], F32)
        ebi = state_p.tile([R, 1], U32)
        ebf = state_p.tile([R, 1], F32)
        ebn = state_p.tile([R, 1], F32)
        ebnu = state_p.tile([R, 1], U32)
        rinvu = state_p.tile([R, 1], U32)

        nc.vector.memset(bufA[:, :], 0.0)
        nc.vector.memset(bufB[:, :], 0.0)
        nc.vector.memset(racc[:, :], 0.0)

        # init: x = (pblock0 + EPS) * ind
        nc.vector.scalar_tensor_tensor(bufA[:, 2:2 + W], phase[0][:, 0:W],
                                       EPS, ind[:, :], ALU.add, ALU.mult)

        bufs = (bufA, bufB)
        scale_slots = {k + 2 for k in RENORM_SLOTS}
        for k in range(NSLOT):
            src_b = bufs[k % 2]
            dst_b = bufs[1 - k % 2]
            blk = k + 1
            ph, bi = divmod(blk, TL)
            psl = phase[ph][:, bi * W: bi * W + FD]
            t_t = tmp_p.tile([R, FD], BF16, tag="t")
            u_t = tmp_p.tile([R, FD], BF16, tag="u")
            w_t = tmp_p.tile([R, FD], BF16, tag="w")
            nc.vector.tensor_tensor(t_t[:, :], src_b[:, 0:FD],
                                    msrc[:, 0:FD], ALU.mult)
            nc.vector.tensor_tensor(u_t[:, :], src_b[:, 2:2 + FD],
                                    src_b[:, 1:1 + FD], ALU.add)
            nc.vector.tensor_tensor(w_t[:, :], u_t[:, :], t_t[:, :], ALU.add)
            if k in scale_slots:
                # fold the pending renorm scale into the p-multiply (EPS is
                # skipped on these 12 slots: ln((p+EPS)/p) <= ~0.1 nat each,
                # negligible vs the 2e-2 rel tolerance on ~1500-nat losses)
                nc.vector.scalar_tensor_tensor(dst_b[:, 2:2 + FD], w_t[:, :],
                                               rinvu[:, :].bitcast(F32), psl,
                                               ALU.mult, ALU.mult)
            else:
                # dst = (p + EPS) * w, mirroring the reference's log(p+EPS)
                nc.vector.scalar_tensor_tensor(dst_b[:, 2:2 + FD], psl,
                                               EPS, w_t[:, :],
                                               ALU.add, ALU.mult)
            if k in RENORM_SLOTS:
                nc.vector.tensor_reduce(mx[:, :], dst_b[:, 2:2 + FD],
                                        axis=mybir.AxisListType.X, op=ALU.max)
                nc.vector.tensor_scalar(ebi[:, :],
                                        mx[:, :].bitcast(U32),
                                        23, None, ALU.logical_shift_right)
                nc.vector.tensor_copy(ebf[:, :], ebi[:, :])
                if V1_RENORM:
                    nc.vector.tensor_scalar(ebf[:, :], ebf[:, :], RT_B, None,
                                            ALU.subtract)
                    nc.vector.tensor_tensor(racc[:, :], racc[:, :], ebf[:, :],
                                            ALU.add)
                    nc.scalar.activation(rinvu[:, :].bitcast(F32), ebf[:, :],
                                         AF.Exp, scale=-LN2)
                else:
                    # DVE-only renorm bookkeeping via fp32 exponent-field
                    # math: eb = expbits(mx); scale 2^(RT_B-127-eb+127)
                    # applied at slot k+2; racc accumulates eb-RT_B,
                    # corrected by LN2 at the end.
                    nc.vector.scalar_tensor_tensor(racc[:, :], ebf[:, :],
                                                   RT_B, racc[:, :],
                                                   ALU.subtract, ALU.add)
                    nc.vector.tensor_scalar(ebn[:, :], ebf[:, :], -1.0, RT_2B,
                                            ALU.mult, ALU.add)
                    nc.vector.tensor_copy(ebnu[:, :], ebn[:, :])
                    nc.vector.tensor_scalar(rinvu[:, :], ebnu[:, :], 23, None,
                                            ALU.logical_shift_left)

        fin = bufs[NSLOT % 2]          # holds a_127 (fwd) / g_128 (bwd)

        # ---- combine ----
        zt = tmp_p.tile([SPC, FD], BF16, tag="t")
        zu = tmp_p.tile([SPC, FD], BF16, tag="u")
        zz = tmp_p.tile([SPC, FD], BF16, tag="w")
        nc.vector.tensor_tensor(zt[:, :], fin[0:SPC, 0:FD],
                                msrc[0:SPC, 0:FD], ALU.mult)
        nc.vector.tensor_tensor(zu[:, :], fin[0:SPC, 2:2 + FD],
                                fin[0:SPC, 1:1 + FD], ALU.add)
        nc.vector.tensor_tensor(zz[:, :], zu[:, :], zt[:, :], ALU.add)

        grev = state_p.tile([SPC, S], BF16)
        raccB = state_p.tile([SPC, 1], F32)
        # reversed copy of bwd rows into fwd partitions: grev[b, s] = g[b, 128-s]
        nc.sync.dma_start(grev[:, :], fin[SPC:R, 2 + 128: 2 - 1: -1])
        nc.sync.dma_start(raccB[:, :], racc[SPC:R, :])

        # log-space combine with exponent/mantissa decomposition (the ACT
        # Ln LUT clamps below ~1e-20, so ln args must stay in [1, 2)):
        #   ln v = LN2 * (expbits(v) - 127) + Ln(mantissa(v))
        # zero entries get a -1e18 penalty so they drop out of logsumexp.
        def exact_ln(src_bf16, pname):
            ebu = state_p.tile([SPC, S], mybir.dt.uint16, name=f"{pname}_ebu")
            ebv = state_p.tile([SPC, S], F32, name=f"{pname}_eb")
            mnt = state_p.tile([SPC, S], mybir.dt.uint16, name=f"{pname}_mn")
            lnm = state_p.tile([SPC, S], F32, name=f"{pname}_lnm")
            pen = state_p.tile([SPC, S], F32, name=f"{pname}_pen")
            lnv = state_p.tile([SPC, S], F32, name=f"{pname}_ln")
            bits = src_bf16.bitcast(mybir.dt.uint16)
            nc.vector.tensor_scalar(ebu[:, :], bits, 7, None,
                                    ALU.logical_shift_right)
            nc.vector.tensor_copy(ebv[:, :], ebu[:, :])
            nc.vector.tensor_scalar(mnt[:, :], bits, 0x7F, None,
                                    ALU.bitwise_and)
            nc.vector.tensor_scalar(mnt[:, :], mnt[:, :], 0x3F80, None,
                                    ALU.bitwise_or)
            nc.scalar.activation(lnm[:, :], mnt[:, :].bitcast(BF16), AF.Ln)
            nc.vector.tensor_scalar(pen[:, :], bits, 0, -1e18,
                                    ALU.is_equal, ALU.mult)
            nc.vector.tensor_scalar(ebv[:, :], ebv[:, :], LN2, -127.0 * LN2,
                                    ALU.mult, ALU.add)
            nc.vector.tensor_tensor(lnv[:, :], ebv[:, :], lnm[:, :], ALU.add)
            nc.vector.tensor_tensor(lnv[:, :], lnv[:, :], pen[:, :], ALU.add)
            return lnv

        lnz = exact_ln(zz[:, 0:S], "z")
        lng = exact_ln(grev[:, :], "g")
        sums = state_p.tile([SPC, S], F32)
        m_t = state_p.tile([SPC, 1], F32)
        negm = state_p.tile([SPC, 1], F32)
        e_t = state_p.tile([SPC, S], F32)
        dot = state_p.tile([SPC, 1], F32)
        nc.vector.tensor_tensor(sums[:, :], lnz[:, :], lng[:, :], ALU.add)
        nc.vector.tensor_scalar_max(sums[:, :], sums[:, :], -1e18)
        nc.vector.tensor_reduce(m_t[:, :], sums[:, :],
                                axis=mybir.AxisListType.X, op=ALU.max)
        nc.vector.tensor_scalar_mul(negm[:, :], m_t[:, :], -1.0)
        nc.scalar.activation(e_t[:, :], sums[:, :], AF.Exp, bias=negm[:, :])
        nc.vector.tensor_reduce(dot[:, :], e_t[:, :],
                                axis=mybir.AxisListType.X, op=ALU.add)
        lnd = state_p.tile([SPC, 1], F32)
        nc.scalar.activation(lnd[:, :], dot[:, :], AF.Ln)
        # loss = -(ln(dot) + m + LN2*(raccF + raccB))
        s1 = state_p.tile([SPC, 1], F32)
        s2 = state_p.tile([SPC, 1], F32)
        s3 = state_p.tile([SPC, 1], F32)
        loss = state_p.tile([SPC, 1], F32)
        nc.vector.tensor_tensor(s1[:, :], racc[0:SPC, :], raccB[:, :], ALU.add)
        nc.vector.tensor_scalar(s1[:, :], s1[:, :], LN2, None, ALU.mult)
        nc.vector.tensor_tensor(s2[:, :], lnd[:, :], m_t[:, :], ALU.add)
        nc.vector.tensor_tensor(s3[:, :], s2[:, :], s1[:, :], ALU.add)
        nc.vector.tensor_scalar_mul(loss[:, :], s3[:, :], -1.0)
        nc.sync.dma_start(out_d.ap(), loss[:, :])
    return nc


_NC_CACHE = None


def _build():
    global _NC_CACHE
    if _NC_CACHE is None:
        nc = bacc.Bacc("TRN2", target_bir_lowering=False, debug=False,
                       enable_asserts=False)
        _emit(nc)
        nc.compile()
        _NC_CACHE = nc
    return _NC_CACHE


def _in_map(y_true_core, y_pred_core):
    idxF, idxB, msrc, ind = _core_tables(y_true_core)
    m = dict(yp=y_pred_core, msrc=msrc, ind=ind)
    for h in range(4):
        m[f"idxf{h}"] = idxF[h]
        m[f"idxb{h}"] = idxB[h]
    return m


def kernel(y_true, y_pred):
    y_true = np.asarray(y_true)
    y_pred = np.ascontiguousarray(np.asarray(y_pred, np.float32))
    nc = _build()
    in_maps = []
    for c in range(NCORES):
        sl = slice(c * SPC, (c + 1) * SPC)
        in_maps.append(_in_map(y_true[sl], np.ascontiguousarray(y_pred[sl])))
    res = run_bass_kernel_spmd(nc, in_maps, core_ids=list(range(NCORES)))
    loss = np.concatenate([res.results[c]["loss_out"] for c in range(NCORES)],
                          axis=0)
    return loss.astype(np.float32)



# revision 23
# speedup vs baseline: 1.0007x; 1.0007x over previous
"""Trainium2 Bass kernel for Keras-style CTC batch loss.

Problem: y_pred [256, 256, 512] f32 softmax probs, y_true [256, 64] int64
labels (0=pad, blank=511). Output [256, 1] f32 negative log-likelihood.

Strategy (pure data parallel, 32 samples per core on 8 cores):
  - Stream y_pred in 16 loads of 1 MB: stage [128 part = 8 samples x 16
    chunks, 4 t x 512 c + zero pad]; every partition line is a contiguous
    8 KB HBM read (bwd t-reversal handled by the pstash-write AP + idx
    tables). fwd loads ride the SP HWDGE ring, bwd loads the ACT HWDGE
    ring, so the two rings drain in parallel at HBM rate.
  - GPSIMD ap_gather pulls 4x144 extended-state probabilities per
    partition (per-core idx = one sample, shared across its 16 chunk
    partitions); ACT casts to bf16 with the reference's +1e-7 fused; a
    collapse DMA lands rows into a [64 rows, t-major] probability store
    (2 tiles of 64 t-blocks each).
  - DVE runs the serial CTC recursion in linear space: 127 slots x 4
    bf16 tensor_tensor ops on [64, 132]. Rows 0..31: forward alpha from
    t=0..127. Rows 32..63: backward adjoint from t=255..128 with the
    state axis flipped about s=128 so both directions share identical
    shifted-view ops. Renorm bookkeeping is pure DVE (fused row-max via
    tensor_tensor_reduce + exponent-field bit math), so the recursion
    never waits on the ACT queue and overlaps the streaming phase.
  - Combine: one extra band apply on the forward rows, reversed dot with
    the backward rows, ln on ACT, negate, DMA out.

Self-contained: shapes/sharding hardcoded; no problem files read.
"""

import numpy as np
import ml_dtypes
from contextlib import ExitStack

import concourse.bass as bass
import concourse.tile as tile
from concourse import bacc, mybir
from concourse.bass_utils import run_bass_kernel_spmd

# ---------------- problem constants ----------------
B, T, C, L = 256, 256, 512, 64
S = 2 * L + 1          # 129 extended states
BLANK = C - 1          # 511
EPS = 1e-7
NCORES = 8
SPC = B // NCORES      # 32 samples per core
R = 2 * SPC            # 64 recursion rows (fwd + bwd)
W = 144                # gather width per t-step (mult of 16, >= S)
FD = 132               # recursion free size (states j = 0..131)
NSLOT = 127            # recursion slots
TQ = 4                 # t-steps packed per stage partition
NCHUNK = 16            # chunk partitions per sample in a stage
TL = TQ * NCHUNK       # 64 t-steps per load
NLOAD = 16             # 8 fwd + 8 bwd
ZCOL = TQ * C          # stage column with exact 0.0 (2048)
NELEM = ZCOL + 4       # stage width incl zero columns (2052)
W4 = TQ * W            # gather idxs per partition (576)
NPH = 2                # phase tiles per direction-merged store
PHW = TL * W           # phase tile width (9216)
RENORM_EVERY = 10
RENORM_SLOTS = tuple(range(3, NSLOT - 2, RENORM_EVERY))
# renorm target 2^36 (biased exp 163): must stay low enough that the FIRST
# renorm's scale 2^(163-eb) is f32-representable (init alpha ~1e-11 -> eb~91
# -> scale 2^72); 10-slot decay (~30 decades) keeps max ~7e-20, far above
# bf16 flush-to-zero even with ~13 decades of state spread.
RT_B = 163.0
RT_2B = 290.0
V1_RENORM = False  # debug: use the v1 ACT-exp renorm path
DEBUG_PH = False   # debug: dump phase stores to dram
LN2 = float(np.log(2.0))
F32 = mybir.dt.float32
BF16 = mybir.dt.bfloat16
I16 = mybir.dt.int16
U32 = mybir.dt.uint32
AF = mybir.ActivationFunctionType
ALU = mybir.AluOpType


# ---------------- host-side tables ----------------
def _ext_and_mask(labels):
    l = int(np.count_nonzero(labels))
    ext = np.full(S, BLANK, np.int64)
    ext[1::2] = np.asarray(labels, np.int64)
    m = np.zeros(S, np.float32)
    mm = (ext[2:] != BLANK) & (ext[2:] != ext[:-2])
    m[2:] = mm.astype(np.float32)
    return ext, m, l


def _wrap576(idx_row):
    """[W4] -> ap_gather wrapped layout [16, W4//16] for one core."""
    return np.asarray(idx_row, np.int16).reshape(W4 // 16, 16).T


def _core_tables(y_true_core):
    """Per-core device tables from labels [SPC, L].

    idxF/idxB: 4 tables [128, W4//16] int16 each (per 8-sample h-group);
    msrc [R, W] bf16 source-side skip mask; ind [R, W] bf16 init flags.
    """
    # per-sample single-t idx rows (within one 513-ish sub-table of 512 + zero)
    fwd_rows = np.full((SPC, W), ZCOL, np.int64)   # offsets into [0, NELEM)
    bwd_rows = np.full((SPC, W), ZCOL, np.int64)
    msrc = np.zeros((R, W), np.float32)
    ind = np.zeros((R, W), np.float32)
    for b in range(SPC):
        ext, m, l = _ext_and_mask(y_true_core[b])
        fwd_rows[b, :2 * l + 1] = ext[:2 * l + 1]
        msrc[b, : S - 2] = m[2:]
        ind[b, 0] = 1.0
        ind[b, 1] = 1.0
        r = SPC + b
        for i in range(S):
            s = 128 - i
            if s <= 2 * l:
                bwd_rows[b, i] = ext[s]
        sidx = 128 - np.arange(W)
        ok = (sidx >= 2) & (sidx <= S - 1)
        msrc[r, ok] = m[sidx[ok]]
        ind[r, 128 - 2 * l] = 1.0
        ind[r, 129 - 2 * l] = 1.0

    def tables(rows, rev):
        # rev: bwd stages hold 4 ascending t per partition while the stream
        # order is t-descending, so sub-position tr reads sub-table 3-tr.
        out = []
        for h in range(4):
            t = np.zeros((128, W4 // 16), np.int16)
            for g in range(8):
                b = 8 * h + g
                full = np.concatenate(
                    [np.where(rows[b] < ZCOL,
                              rows[b] + (TQ - 1 - tr if rev else tr) * C,
                              ZCOL)
                     for tr in range(TQ)])
                t[16 * g: 16 * g + 16, :] = _wrap576(full)
            out.append(t)
        return out

    return (tables(fwd_rows, False), tables(bwd_rows, True),
            msrc.astype(ml_dtypes.bfloat16),
            ind.astype(ml_dtypes.bfloat16))


# ---------------- device kernel ----------------
def _emit(nc):
    yp = nc.dram_tensor("yp", [SPC, T, C], F32, kind="ExternalInput")
    idxf_d = [nc.dram_tensor(f"idxf{h}", [128, W4 // 16], I16,
                             kind="ExternalInput") for h in range(4)]
    idxb_d = [nc.dram_tensor(f"idxb{h}", [128, W4 // 16], I16,
                             kind="ExternalInput") for h in range(4)]
    msrc_d = nc.dram_tensor("msrc", [R, W], BF16, kind="ExternalInput")
    ind_d = nc.dram_tensor("ind", [R, W], BF16, kind="ExternalInput")
    out_d = nc.dram_tensor("loss_out", [SPC, 1], F32, kind="ExternalOutput")
    # DRAM staging for the gathered probabilities, f32 (no dtype cast
    # anywhere: DMA-cast runs ~16us/write). Writing straight into the SBUF
    # phase tiles WAW-chains 8 DMAs per tile at the ~15.7us SWDGE completion
    # latency each; through DRAM the 16 writes are independent and each
    # phase needs one read that blocks only once on the last write.
    pstash = nc.dram_tensor("pstash", [2 * NPH * 4 * 128, W4], F32,
                            kind="Internal")

    with tile.TileContext(nc) as tc, ExitStack() as ctx:
        consts = ctx.enter_context(tc.tile_pool(name="consts", bufs=1))
        stage_p = ctx.enter_context(tc.tile_pool(name="stage", bufs=1))
        # 8 gout bufs: gather_i's WAR waits write_{i-8}'s DMA completion
        # (~15.7us after issue), which at ~2.8us/load cadence is already met
        gout_p = ctx.enter_context(tc.tile_pool(name="gout", bufs=8))
        phase_p = ctx.enter_context(tc.tile_pool(name="phase", bufs=1))
        state_p = ctx.enter_context(tc.tile_pool(name="state", bufs=1))
        tmp_p = ctx.enter_context(tc.tile_pool(name="tmp", bufs=3))

        idxf = [consts.tile([128, W4 // 16], I16, name=f"idxf{h}")
                for h in range(4)]
        idxb = [consts.tile([128, W4 // 16], I16, name=f"idxb{h}")
                for h in range(4)]
        msrc = consts.tile([R, W], BF16)
        ind = consts.tile([R, W], BF16)
        # consts ride the gpsimd SWDGE queue so the sync/scalar HWDGE rings
        # start pulling y_pred immediately
        for h in range(4):
            nc.gpsimd.dma_start(idxf[h][:, :], idxf_d[h].ap())
            nc.gpsimd.dma_start(idxb[h][:, :], idxb_d[h].ap())
        nc.gpsimd.dma_start(msrc[:, :], msrc_d.ap())
        nc.gpsimd.dma_start(ind[:, :], ind_d.ap())

        # 12 stages (vs 16 loads) to fit f32 phase tiles in SBUF; loads
        # 12..15 reuse stages 0..3 and are emitted only after those stages'
        # gathers (program-order WAR).
        NSTAGE = 12
        stages = [stage_p.tile([128, NELEM], F32, name=f"stage{i}")
                  for i in range(NSTAGE)]
        for st in stages:
            nc.vector.memset(st[:, ZCOL:NELEM], 0.0)

        # f32 phase stores; the recursion multiplies read f32 psl directly
        phase = [phase_p.tile([R, PHW], F32, name=f"phase{j}")
                 for j in range(NPH)]

        # loads: (is_bwd, j, h); order fwd/bwd interleaved, j-major
        loads = []
        for j in range(NPH):
            for h in range(4):
                loads.append((0, j, h))
                loads.append((1, j, h))

        def src_ap(is_bwd, j, h):
            # Always a contiguous, t-ascending chunk range: 8KB/partition HBM
            # lines (the SDMA fast path). The bwd t-reversal happens in the
            # pstash-write AP (chunk dim) + idx tables (sub-t).
            blocks = yp.ap()[8 * h: 8 * h + 8, :, :].rearrange(
                "b (k t) c -> b k (t c)", t=TQ)
            lo = (T // TQ - NCHUNK * (j + 1)) if is_bwd else NCHUNK * j
            return blocks[:, lo: lo + NCHUNK, :]

        def issue_load(i):
            # fwd loads on the SP HWDGE ring, bwd on the ACT HWDGE ring: the
            # two rings drain in parallel, bounded only by HBM bandwidth.
            d, j, h = loads[i]
            eng = nc.scalar if d else nc.sync
            eng.dma_start(stages[i % NSTAGE][:, 0:ZCOL], src_ap(d, j, h))

        for i in range(NSTAGE):
            issue_load(i)

        for i, (d, j, h) in enumerate(loads):
            st = stages[i % NSTAGE]
            go = gout_p.tile([128, W4], F32, tag="gout")
            idx = (idxb if d else idxf)[h]
            nc.gpsimd.ap_gather(
                out_ap=go[:, :], in_ap=st[:, :], idxs_ap=idx[:, :],
                channels=128, num_elems=NELEM, d=1, num_idxs=W4)
            if i + NSTAGE < len(loads):
                # the load reusing this stage must be emitted AFTER its
                # gather so the WAR is in program order
                issue_load(i + NSTAGE)
            if DEBUG_PH and i == 0:
                dbg_st = nc.dram_tensor("dbg_st", [128, NELEM], F32,
                                        kind="ExternalOutput")
                dbg_go = nc.dram_tensor("dbg_go", [128, W4], F32,
                                        kind="ExternalOutput")
                nc.sync.dma_start(dbg_st.ap(), st[:, :])
                nc.sync.dma_start(dbg_go.ap(), go[:, :])
            # stash to DRAM: src partition (g, c) -> pstash row (g, c) for
            # fwd, (g, 15-c) for bwd (t-reversal: stage chunk c holds
            # t-block lo+c ascending, phase col p needs t-block 63-16j-p =
            # stage chunk 15-p). Same gpsimd queue as the gather, so the
            # write needs no cross-engine wait.
            base = ((d * NPH + j) * 4 + h) * 128
            dst = pstash.ap()[base: base + 128, :]
            if d:
                dst = dst.rearrange("(g c) s -> g c s", c=NCHUNK)[:, ::-1, :]
            nc.gpsimd.dma_start(dst, go[:, :])
            if i == 7 or i == 15:
                # phase j gather-in right after its 8 pstash writes
                jj = i // 8
                a = pstash.ap().rearrange("(d j q) s -> d j q s", d=2, j=NPH)
                psrc = a[:, jj:jj + 1, :, :].squeeze(1).rearrange(
                    "d (q c) s -> d q (c s)", c=NCHUNK)
                nc.sync.dma_start(phase[jj][:, :], psrc)

        if DEBUG_PH:
            dbg = [nc.dram_tensor(f"dbg_ph{j}", [R, PHW], F32,
                                  kind="ExternalOutput") for j in range(NPH)]
            for j in range(NPH):
                nc.sync.dma_start(dbg[j].ap(), phase[j][:, :])

        # ---- recursion state ----
        bufA = state_p.tile([R, 2 + W], BF16)
        bufB = state_p.tile([R, 2 + W], BF16)
        racc = state_p.tile([R, 1], F32)
        mx = state_p.tile([R, 1], F32)
        ebi = state_p.tile([R, 1], U32)
        ebf = state_p.tile([R, 1], F32)
        ebn = state_p.tile([R, 1], F32)
        ebnu = state_p.tile([R, 1], U32)
        rinvu = state_p.tile([R, 1], U32)

        nc.vector.memset(bufA[:, :], 0.0)
        nc.vector.memset(bufB[:, :], 0.0)
        nc.vector.memset(racc[:, :], 0.0)

        # init: x = (pblock0 + EPS) * ind
        nc.vector.scalar_tensor_tensor(bufA[:, 2:2 + W], phase[0][:, 0:W],
                                       EPS, ind[:, :], ALU.add, ALU.mult)

        bufs = (bufA, bufB)
        scale_slots = {k + 2 for k in RENORM_SLOTS}
        for k in range(NSLOT):
            src_b = bufs[k % 2]
            dst_b = bufs[1 - k % 2]
            blk = k + 1
            ph, bi = divmod(blk, TL)
            psl = phase[ph][:, bi * W: bi * W + FD]
            t_t = tmp_p.tile([R, FD], BF16, tag="t")
            u_t = tmp_p.tile([R, FD], BF16, tag="u")
            w_t = tmp_p.tile([R, FD], BF16, tag="w")
            nc.vector.tensor_tensor(t_t[:, :], src_b[:, 0:FD],
                                    msrc[:, 0:FD], ALU.mult)
            nc.vector.tensor_tensor(u_t[:, :], src_b[:, 2:2 + FD],
                                    src_b[:, 1:1 + FD], ALU.add)
            nc.vector.tensor_tensor(w_t[:, :], u_t[:, :], t_t[:, :], ALU.add)
            if k in scale_slots:
                # fold the pending renorm scale into the p-multiply (EPS is
                # skipped on these 12 slots: ln((p+EPS)/p) <= ~0.1 nat each,
                # negligible vs the 2e-2 rel tolerance on ~1500-nat losses)
                nc.vector.scalar_tensor_tensor(dst_b[:, 2:2 + FD], w_t[:, :],
                                               rinvu[:, :].bitcast(F32), psl,
                                               ALU.mult, ALU.mult)
            else:
                # dst = (p + EPS) * w, mirroring the reference's log(p+EPS).
                # At renorm slots the row-sum rides along as accum_out: the
                # renorm scale only needs a magnitude proxy (sum is within
                # 7 bits of max; racc/rinvu stay exactly consistent).
                nc.vector.scalar_tensor_tensor(
                    dst_b[:, 2:2 + FD], psl, EPS, w_t[:, :],
                    ALU.add, ALU.mult,
                    accum_out=mx[:, :] if k in RENORM_SLOTS else None)
            if k in RENORM_SLOTS:
                nc.vector.tensor_scalar(ebi[:, :],
                                        mx[:, :].bitcast(U32),
                                        23, None, ALU.logical_shift_right)
                nc.vector.tensor_copy(ebf[:, :], ebi[:, :])
                if V1_RENORM:
                    nc.vector.tensor_scalar(ebf[:, :], ebf[:, :], RT_B, None,
                                            ALU.subtract)
                    nc.vector.tensor_tensor(racc[:, :], racc[:, :], ebf[:, :],
                                            ALU.add)
                    nc.scalar.activation(rinvu[:, :].bitcast(F32), ebf[:, :],
                                         AF.Exp, scale=-LN2)
                else:
                    # DVE-only renorm bookkeeping via fp32 exponent-field
                    # math: eb = expbits(mx); scale 2^(RT_B-127-eb+127)
                    # applied at slot k+2; racc accumulates eb-RT_B,
                    # corrected by LN2 at the end.
                    nc.vector.scalar_tensor_tensor(racc[:, :], ebf[:, :],
                                                   RT_B, racc[:, :],
                                                   ALU.subtract, ALU.add)
                    nc.vector.tensor_scalar(ebn[:, :], ebf[:, :], -1.0, RT_2B,
                                            ALU.mult, ALU.add)
                    nc.vector.tensor_copy(ebnu[:, :], ebn[:, :])
                    nc.vector.tensor_scalar(rinvu[:, :], ebnu[:, :], 23, None,
                                            ALU.logical_shift_left)

        fin = bufs[NSLOT % 2]          # holds a_127 (fwd) / g_128 (bwd)

        # ---- combine ----
        zt = tmp_p.tile([SPC, FD], BF16, tag="t")
        zu = tmp_p.tile([SPC, FD], BF16, tag="u")
        zz = tmp_p.tile([SPC, FD], BF16, tag="w")
        nc.vector.tensor_tensor(zt[:, :], fin[0:SPC, 0:FD],
                                msrc[0:SPC, 0:FD], ALU.mult)
        nc.vector.tensor_tensor(zu[:, :], fin[0:SPC, 2:2 + FD],
                                fin[0:SPC, 1:1 + FD], ALU.add)
        nc.vector.tensor_tensor(zz[:, :], zu[:, :], zt[:, :], ALU.add)

        grev = state_p.tile([SPC, S], BF16)
        raccB = state_p.tile([SPC, 1], F32)
        # reversed copy of bwd rows into fwd partitions: grev[b, s] = g[b, 128-s]
        nc.sync.dma_start(grev[:, :], fin[SPC:R, 2 + 128: 2 - 1: -1])
        nc.sync.dma_start(raccB[:, :], racc[SPC:R, :])

        # Direct linear-space dot, DVE-only (no ACT round trips: blocking
        # cross-engine engine-sem waits cost ~15.7us each). zz*2^48*grev
        # stays within f32 range: zz,grev <= ~2^38, products <= ~4e36;
        # post-renorm decay keeps the scaled dot above f32 flush-to-zero.
        SC48 = float(2.0 ** 48)
        prod = state_p.tile([SPC, S], F32)
        dot = state_p.tile([SPC, 1], F32)
        nc.vector.scalar_tensor_tensor(prod[:, :], zz[:, 0:S], SC48,
                                       grev[:, :], ALU.mult, ALU.mult)
        nc.vector.tensor_reduce(dot[:, :], prod[:, :],
                                axis=mybir.AxisListType.X, op=ALU.add)
        # ln(dot) via exponent field + atanh-series mantissa ln:
        #   m in [1,2), z = (m-1)/(m+1) <= 1/3,
        #   ln m = 2z(1 + z^2/3 + z^4/5) + O(2e-4)
        ebu2 = state_p.tile([SPC, 1], U32)
        ebf2 = state_p.tile([SPC, 1], F32)
        mnt2 = state_p.tile([SPC, 1], U32)
        num = state_p.tile([SPC, 1], F32)
        den = state_p.tile([SPC, 1], F32)
        rden = state_p.tile([SPC, 1], F32)
        z_t = state_p.tile([SPC, 1], F32)
        z2 = state_p.tile([SPC, 1], F32)
        p1 = state_p.tile([SPC, 1], F32)
        q_t = state_p.tile([SPC, 1], F32)
        lnm = state_p.tile([SPC, 1], F32)
        nc.vector.tensor_scalar(ebu2[:, :], dot[:, :].bitcast(U32), 23, None,
                                ALU.logical_shift_right)
        nc.vector.tensor_copy(ebf2[:, :], ebu2[:, :])
        nc.vector.tensor_scalar(mnt2[:, :], dot[:, :].bitcast(U32),
                                0x7FFFFF, 0x3F800000, ALU.bitwise_and,
                                ALU.bitwise_or)
        m_f = mnt2[:, :].bitcast(F32)
        nc.vector.tensor_scalar(num[:, :], m_f, -1.0, None, ALU.add)
        nc.vector.tensor_scalar(den[:, :], m_f, 1.0, None, ALU.add)
        nc.vector.reciprocal(rden[:, :], den[:, :])
        nc.vector.tensor_tensor(z_t[:, :], num[:, :], rden[:, :], ALU.mult)
        nc.vector.tensor_tensor(z2[:, :], z_t[:, :], z_t[:, :], ALU.mult)
        nc.vector.tensor_scalar(p1[:, :], z2[:, :], 0.2, 1.0 / 3.0,
                                ALU.mult, ALU.add)
        nc.vector.tensor_tensor(q_t[:, :], z2[:, :], p1[:, :], ALU.mult)
        # lnm = (q + 1) * z
        nc.vector.scalar_tensor_tensor(lnm[:, :], q_t[:, :], 1.0, z_t[:, :],
                                       ALU.add, ALU.mult)
        # loss = -(2*lnm + LN2*(eb - 127 - 48 + raccF + raccB))
        s1 = state_p.tile([SPC, 1], F32)
        s2 = state_p.tile([SPC, 1], F32)
        s3 = state_p.tile([SPC, 1], F32)
        loss = state_p.tile([SPC, 1], F32)
        nc.vector.tensor_tensor(s1[:, :], racc[0:SPC, :], raccB[:, :], ALU.add)
        nc.vector.tensor_tensor(s2[:, :], s1[:, :], ebf2[:, :], ALU.add)
        nc.vector.tensor_scalar(s2[:, :], s2[:, :], LN2, -175.0 * LN2,
                                ALU.mult, ALU.add)
        nc.vector.scalar_tensor_tensor(s3[:, :], lnm[:, :], 2.0, s2[:, :],
                                       ALU.mult, ALU.add)
        nc.vector.tensor_scalar_mul(loss[:, :], s3[:, :], -1.0)
        nc.sync.dma_start(out_d.ap(), loss[:, :])
    return nc


_NC_CACHE = None


def _build():
    global _NC_CACHE
    if _NC_CACHE is None:
        nc = bacc.Bacc("TRN2", target_bir_lowering=False, debug=False,
                       enable_asserts=False)
        _emit(nc)
        nc.compile()
        _NC_CACHE = nc
    return _NC_CACHE


def _in_map(y_true_core, y_pred_core):
    idxF, idxB, msrc, ind = _core_tables(y_true_core)
    m = dict(yp=y_pred_core, msrc=msrc, ind=ind)
    for h in range(4):
        m[f"idxf{h}"] = idxF[h]
        m[f"idxb{h}"] = idxB[h]
    return m


def kernel(y_true, y_pred):
    y_true = np.asarray(y_true)
    y_pred = np.ascontiguousarray(np.asarray(y_pred, np.float32))
    nc = _build()
    in_maps = []
    for c in range(NCORES):
        sl = slice(c * SPC, (c + 1) * SPC)
        in_maps.append(_in_map(y_true[sl], np.ascontiguousarray(y_pred[sl])))
    res = run_bass_kernel_spmd(nc, in_maps, core_ids=list(range(NCORES)))
    loss = np.concatenate([res.results[c]["loss_out"] for c in range(NCORES)],
                          axis=0)
    return loss.astype(np.float32)

